# revision 27
# baseline (speedup 1.0000x reference)
"""Sparse-attention Trainium2 kernel (8 NeuronCores, data-parallel over N).

Math (per batch element n, derived from the reference):
    Q  = Pn @ Wq.T + bq                      [T, C]
    scores = Q @ K.T  with K = Mn @ Wk.T + bk
           = (Q @ Wk) @ Mn.T + (Q . bk)      -- bk term is constant over the
                                                softmax group (mV axis) -> dropped
    Q' = Q @ Wk                              [T, C]
    sm = scores + additive_mask              (masked lanes == -1e15 exactly in f32)
    attn = softmax over mV groups            (per-t row max as the exp shift --
                                              safe: observed per-row group-max gap
                                              << 87, so no under/overflow)
    out = attn @ V,  V = Mn @ Wv.T + bv
        = (attn @ Mn) @ Wv.T + 32*bv         -- since sum_s attn[t, s] == 32 exactly
    att = mean over (t, tm) of attn          [mV]

Layouts on chip (partition dim first):
    PT   [c', (cc, t)]      via PE transpose of P natural
    QT   [c, (cc, t)]       = WqT.T @ PT    (+bq via ACT bias)
    Q'T  [c', (cc, t)]      = Wk_nat.T @ QT
    Mn   [s, (sc, c)]       natural DMA load (used as lhsT for Z)
    MnT  [c', (cc, s)]      via PE transpose (moving operand for scores)
    scores PSUM [t, s-slice] = Q'T.T @ MnT, + mask via K=1 matmul;
    exp on ACT with bias=-rowmax (fused PSUM->SBUF)
    attn [t, (tc, s)]       normalized on DVE, transposed to attnT [s, (sc, t)]
    ZT   [c', (cc, t)]      = Mn.T @ attnT
    out  [t, (tc, c)]       = ZT.T @ WvT + 32*bv
"""

import sys

for _p in ("/opt/trn_rl_repo",):
    if _p not in sys.path:
        sys.path.insert(0, _p)

from contextlib import ExitStack

import numpy as np

import concourse.bass as bass
from concourse import bacc
import concourse.mybir as mybir
import concourse.tile as tile
from concourse.bass_utils import run_bass_kernel_spmd
from concourse.masks import make_identity

T, N, C = 256, 64, 256
mV, mT = 64, 32
S = mV * mT          # 2048
NCORES = 8
NLOC = N // NCORES   # 8 batch elements per core
TC = T // 128        # 2 t-chunks
CC = C // 128        # 2 c-chunks
SC = S // 128        # 16 s-chunks
SL = S // 512        # 4 s-slices (fp32 moving max)
SG = SC // 4         # 4 transpose groups of 4 chunks

F32 = mybir.dt.float32
F32R = mybir.dt.float32r
BF16 = mybir.dt.bfloat16
AX = mybir.AxisListType
AF = mybir.ActivationFunctionType


def r(ap):
    """Bitcast an AP to float32r for full-rate fp32 matmuls."""
    return ap.bitcast(F32R)


def _build_graph():
    nc = bacc.Bacc()

    P_d = nc.declare_dram_parameter("P", [T, NLOC, C], F32, isOutput=False)
    M_d = nc.declare_dram_parameter("M", [mT, NLOC * mV, C], F32, isOutput=False)
    mk_d = nc.declare_dram_parameter("maskf", [NLOC, S], F32, isOutput=False)
    Wq_d = nc.declare_dram_parameter("Wq", [C, C], F32, isOutput=False)
    bq_d = nc.declare_dram_parameter("bq", [C], F32, isOutput=False)
    Wk_d = nc.declare_dram_parameter("Wk", [C, C], F32, isOutput=False)
    Wv_d = nc.declare_dram_parameter("Wv", [C, C], F32, isOutput=False)
    bv32_d = nc.declare_dram_parameter("bv32", [C], F32, isOutput=False)
    out_d = nc.declare_dram_parameter("out", [T, NLOC, C], F32, isOutput=True)
    att_d = nc.declare_dram_parameter("att", [NLOC, mV], F32, isOutput=True)

    with tile.TileContext(nc) as tc, ExitStack() as ctx:
        const = ctx.enter_context(tc.tile_pool(name="const", bufs=1))
        mn_pool = ctx.enter_context(tc.tile_pool(name="mn", bufs=2))
        mnt_pool = ctx.enter_context(tc.tile_pool(name="mnt", bufs=2))
        attn_pool = ctx.enter_context(tc.tile_pool(name="attn", bufs=2))
        attnt_pool = ctx.enter_context(tc.tile_pool(name="attnt", bufs=2))
        work = ctx.enter_context(tc.tile_pool(name="work", bufs=2))
        outp = ctx.enter_context(tc.tile_pool(name="outp", bufs=3))
        ps_big = ctx.enter_context(tc.tile_pool(name="psbig", bufs=4, space="PSUM"))
        ps_tr = ctx.enter_context(tc.tile_pool(name="pstr", bufs=2, space="PSUM"))
        ps_sm = ctx.enter_context(tc.tile_pool(name="pssm", bufs=2, space="PSUM"))

        # ---- constants ----
        ident = const.tile([128, 128], F32)
        make_identity(nc, ident)
        identb = const.tile([128, 128], BF16)
        make_identity(nc, identb)
        onecol_b = const.tile([128, 1], BF16)
        nc.vector.memset(onecol_b, 1.0)
        ones_raw = const.tile([1, 128], F32)
        nc.vector.memset(ones_raw, 1.0)
        ones1 = const.tile([1, 128], F32)
        nc.scalar.copy(r(ones1), ones_raw)

        # weights: single batched DMA each
        wq_nat = const.tile([128, CC, C], F32)
        wv_nat = const.tile([128, CC, C], F32)
        wk_sb = const.tile([128, CC, C], F32)   # Wk natural [co, (coc, ci)] used as-is
        nc.sync.dma_start(out=wq_nat, in_=Wq_d[:, :].rearrange("(cc p) ci -> p cc ci", p=128))
        nc.sync.dma_start(out=wv_nat, in_=Wv_d[:, :].rearrange("(cc p) ci -> p cc ci", p=128))
        nc.sync.dma_start(out=r(wk_sb), in_=r(Wk_d[:, :].rearrange("(cc p) ci -> p cc ci", p=128)))
        wqt = const.tile([128, CC, C], F32)     # [ci_loc, cic, co]
        wvt = const.tile([128, CC, C], F32)
        for cic in range(CC):
            pst = ps_tr.tile([128, 512], F32, tag="pstr")
            nc.tensor.transpose(pst[:, 0:128], wq_nat[:, 0, cic * 128:(cic + 1) * 128], ident)
            nc.tensor.transpose(pst[:, 128:256], wq_nat[:, 1, cic * 128:(cic + 1) * 128], ident)
            nc.tensor.transpose(pst[:, 256:384], wv_nat[:, 0, cic * 128:(cic + 1) * 128], ident)
            nc.tensor.transpose(pst[:, 384:512], wv_nat[:, 1, cic * 128:(cic + 1) * 128], ident)
            nc.scalar.copy(r(wqt[:, cic, :]), pst[:, 0:256])
            nc.scalar.copy(r(wvt[:, cic, :]), pst[:, 256:512])

        bq_sb = const.tile([128, CC], F32)
        for cc in range(CC):
            nc.sync.dma_start(
                out=bq_sb[:, cc:cc + 1],
                in_=bq_d[cc * 128:(cc + 1) * 128].unsqueeze(1),
            )
        bv32_sb = const.tile([128, C], F32)
        nc.sync.dma_start(out=bv32_sb, in_=bv32_d[:].partition_broadcast(128))

        # ---- software-pipelined per-batch-element stages ----
        # prep(n):  P/M loads, PT/QT/Q'T, MnT transposes  (PE/ACT/DMA)
        # soft(n):  scores, group-max softmax, normalize, attnT  (PE/DVE/ACT)
        # tail(n):  att matmuls, ZT, out  (PE)
        # Emission order interleaves prep(i+2) | soft(i+1)+tail-of-PE work so the
        # PE stream never waits long on the DVE softmax chain.
        state = {}

        def prep(n):
            p_nat = work.tile([128, TC, C], F32, tag="pnat")
            nc.sync.dma_start(
                out=p_nat,
                in_=P_d[:, n, :].rearrange("(tc p) c -> p tc c", p=128),
            )
            pt = work.tile([128, CC, T], F32, tag="pt")
            pstp = ps_tr.tile([128, 512], F32, tag="pstr")
            for tc_i in range(TC):
                for cc in range(CC):
                    nc.tensor.transpose(
                        pstp[:, (cc * TC + tc_i) * 128:(cc * TC + tc_i + 1) * 128],
                        p_nat[:, tc_i, cc * 128:(cc + 1) * 128], ident,
                    )
            for cc in range(CC):
                nc.scalar.copy(r(pt[:, cc, :]), pstp[:, cc * 256:(cc + 1) * 256])

            qt = work.tile([128, CC, T], F32, tag="qt")
            for cc in range(CC):
                psq = ps_sm.tile([128, T], F32, tag="pssm")
                for k in range(CC):
                    nc.tensor.matmul(
                        psq, r(wqt[:, k, cc * 128:(cc + 1) * 128]), r(pt[:, k, :]),
                        start=(k == 0), stop=(k == CC - 1),
                    )
                nc.scalar.add(r(qt[:, cc, :]), psq, add=bq_sb[:, cc:cc + 1])

            qpt = work.tile([128, CC, T], F32, tag="qpt")
            for cc in range(CC):
                psq = ps_sm.tile([128, T], F32, tag="pssm")
                for k in range(CC):
                    nc.tensor.matmul(
                        psq, r(wk_sb[:, k, cc * 128:(cc + 1) * 128]), r(qt[:, k, :]),
                        start=(k == 0), stop=(k == CC - 1),
                    )
                nc.scalar.copy(r(qpt[:, cc, :]), psq)

            # mn partitions are tm-major: p = tm*4 + vl (matches M's DRAM
            # order, giving 32x4KB DMA descriptors instead of 128x1KB)
            mn = mn_pool.tile([128, SC, C], F32, tag="mn")
            for sc in range(SC):
                msrc = M_d[:, n * mV + sc * 4:n * mV + (sc + 1) * 4, :]
                nc.sync.dma_start(out=r(mn[:, sc, :]), in_=r(msrc))
            mnt = mnt_pool.tile([128, CC, S], F32, tag="mnt")
            for cc in range(CC):
                for g in range(SG):
                    pst = ps_tr.tile([128, 512], F32, tag="pstr")
                    for j in range(4):
                        nc.tensor.transpose(
                            pst[:, j * 128:(j + 1) * 128],
                            mn[:, g * 4 + j, cc * 128:(cc + 1) * 128], ident,
                        )
                    nc.scalar.copy(r(mnt[:, cc, g * 512:(g + 1) * 512]), pst)

            mrow = work.tile([1, S], F32, tag="mrow")
            nc.sync.dma_start(out=r(mrow), in_=r(mk_d[n:n + 1, :]))
            state[n] = dict(qpt=qpt, mn=mn, mnt=mnt, mrow=mrow)

        def soft(n):
            st = state[n]
            qpt, mnt, mrow = st["qpt"], st["mnt"], st["mrow"]
            attn_sb = attn_pool.tile([128, TC, S], BF16, tag="attn")
            attnt = attnt_pool.tile([128, SC, T], F32, tag="attnt")
            gvs = work.tile([128, TC, SC], F32, tag="gvs")
            for tc_i in range(TC):
                slices = []
                for k in range(CC):
                    for sl in range(SL):
                        if k == 0:
                            pss = ps_big.tile([128, 512], F32, tag="psbig")
                            slices.append(pss)
                        nc.tensor.matmul(
                            slices[sl],
                            r(qpt[:, k, tc_i * 128:(tc_i + 1) * 128]),
                            r(mnt[:, k, sl * 512:(sl + 1) * 512]),
                            start=(k == 0), stop=False,
                        )
                pm = work.tile([128, SL, mT], F32, tag="pm")
                for sl in range(SL):
                    nc.tensor.matmul(
                        slices[sl], r(ones1), r(mrow[:, sl * 512:(sl + 1) * 512]),
                        start=False, stop=True,
                    )
                    nc.vector.reduce_max(
                        out=pm[:, sl, :],
                        in_=slices[sl][:, :]
                        .rearrange("p (scl tm vl) -> p tm scl vl", tm=mT, vl=4),
                        axis=AX.XY,
                    )
                mx = work.tile([128, mT], F32, tag="mx")
                nc.vector.reduce_max(
                    out=mx, in_=pm.rearrange("p sl tm -> p tm sl"), axis=AX.X,
                )
                sc_f = work.tile([128, S], F32, tag="scf")
                for sl in range(SL):
                    nc.vector.tensor_sub(
                        sc_f[:, sl * 512:(sl + 1) * 512]
                        .rearrange("p (scl tm vl) -> p scl tm vl", tm=mT, vl=4),
                        slices[sl][:, :]
                        .rearrange("p (scl tm vl) -> p scl tm vl", tm=mT, vl=4),
                        mx.unsqueeze(1).unsqueeze(3).broadcast_to([128, 4, mT, 4]),
                    )
                    nc.scalar.activation(
                        attn_sb[:, tc_i, sl * 512:(sl + 1) * 512],
                        sc_f[:, sl * 512:(sl + 1) * 512], AF.Exp,
                        bias=0.0, scale=1.0,
                    )
                # denominator: contiguous binary-tree sum over the mV axis
                half = S // 2
                dtree = work.tile([128, half], BF16, tag="dtree")
                nc.vector.tensor_add(
                    dtree[:, 0:half],
                    attn_sb[:, tc_i, 0:half], attn_sb[:, tc_i, half:S],
                )
                w = half // 2
                while w >= 128:
                    nc.vector.tensor_add(
                        dtree[:, 0:w], dtree[:, 0:w], dtree[:, w:2 * w],
                    )
                    w //= 2
                denom = work.tile([128, mT], F32, tag="denom")
                nc.vector.reduce_sum(
                    out=denom,
                    in_=dtree[:, 0:128].rearrange("p (tm vl) -> p tm vl", vl=4),
                    axis=AX.X,
                )
                recip = work.tile([128, mT], BF16, tag="recip")
                with nc.allow_low_precision(reason="attn weights <= 1; bf16 ok"):
                    nc.vector.reciprocal(recip, denom)
                nc.vector.tensor_mul(
                    attn_sb[:, tc_i, :]
                    .rearrange("p (sc tm vl) -> p sc tm vl", tm=mT, vl=4),
                    attn_sb[:, tc_i, :]
                    .rearrange("p (sc tm vl) -> p sc tm vl", tm=mT, vl=4),
                    recip.unsqueeze(1).unsqueeze(3).broadcast_to([128, SC, mT, 4]),
                )
                # transpose attn -> attnT [s, (sc, t)] (batches of 8 blocks)
                for g in range(2):
                    pstb = ps_big.tile([128, 8, 128], BF16, tag="psbig")
                    for j in range(8):
                        nc.tensor.transpose(
                            pstb[:, j, :],
                            attn_sb[:, tc_i, (g * 8 + j) * 128:(g * 8 + j + 1) * 128],
                            identb,
                        )
                    nc.scalar.copy(
                        r(attnt[:, g * 8:(g + 1) * 8, tc_i * 128:(tc_i + 1) * 128]),
                        pstb,
                    )
                # att partial sums over t via PE: ones-column matmuls per s-chunk
                psat = ps_sm.tile([128, SC], F32, tag="pssm")
                for sc in range(SC):
                    nc.tensor.matmul(
                        psat[:, sc:sc + 1],
                        attn_sb[:, tc_i, sc * 128:(sc + 1) * 128],
                        onecol_b, start=True, stop=True,
                    )
                nc.scalar.copy(gvs[:, tc_i, :], psat)
            st["attnt"] = attnt
            st["gvs"] = gvs

        def tail(n):
            st = state.pop(n)
            mn, attnt, gvs = st["mn"], st["attnt"], st["gvs"]
            # att[v=(sc,vl)] = (1/(T*mT)) * sum_tm rowsums
            rowsum1 = work.tile([128, SC], F32, tag="rowsum1")
            nc.vector.tensor_add(rowsum1, gvs[:, 0, :], gvs[:, 1, :])
            psr = ps_sm.tile([16, 128], F32, tag="pssm")
            nc.tensor.transpose(psr, rowsum1, ident)
            rst = work.tile([16, 128], F32, tag="rst")
            nc.scalar.mul(rst, psr, 1.0 / (T * mT))
            att_fin = outp.tile([16, 4], F32, tag="attfin")
            nc.vector.reduce_sum(
                out=att_fin,
                in_=rst.rearrange("p (tm vl) -> p vl tm", vl=4),
                axis=AX.X,
            )
            nc.sync.dma_start(
                out=att_d[n, :].rearrange("(sc vl) -> sc vl", vl=4), in_=att_fin,
            )

            zt = work.tile([128, CC, T], F32, tag="zt")
            for cc in range(CC):
                psz = ps_sm.tile([128, T], F32, tag="pssm")
                for sc in range(SC):
                    nc.tensor.matmul(
                        psz, r(mn[:, sc, cc * 128:(cc + 1) * 128]), r(attnt[:, sc, :]),
                        start=(sc == 0), stop=(sc == SC - 1),
                    )
                nc.scalar.copy(r(zt[:, cc, :]), psz)

            o_sb = outp.tile([128, TC, C], F32, tag="osb")
            for tc_i in range(TC):
                pso = ps_sm.tile([128, C], F32, tag="pssm")
                for k in range(CC):
                    nc.tensor.matmul(
                        pso, r(zt[:, k, tc_i * 128:(tc_i + 1) * 128]), r(wvt[:, k, :]),
                        start=(k == 0), stop=(k == CC - 1),
                    )
                nc.vector.tensor_add(o_sb[:, tc_i, :], pso, bv32_sb)
            nc.sync.dma_start(
                out=out_d[:, n, :].rearrange("(tc p) c -> p tc c", p=128), in_=o_sb,
            )

        # staggered emission: prep two ahead, softmax one ahead, tail current
        prep(0)
        prep(1)
        soft(0)
        for i in range(NLOC):
            if i + 2 < NLOC:
                prep(i + 2)
            if i + 1 < NLOC:
                soft(i + 1)
            tail(i)

    nc.finalize()
    return nc


_NC_CACHE = {}


def kernel(P, M, mask, Wq, bq, Wk, bk, Wv, bv):
    P = np.ascontiguousarray(P, dtype=np.float32)
    M = np.ascontiguousarray(M, dtype=np.float32)
    maskf = np.where(np.asarray(mask), np.float32(0.0), np.float32(-1e15)).astype(np.float32)
    # permuted on-chip s layout: s = sc*128 + tm*4 + vl  (v = sc*4 + vl)
    maskrow = np.ascontiguousarray(
        np.broadcast_to(
            maskf.reshape(N, SC, 1, 4), (N, SC, mT, 4)
        ).reshape(N, S)
    )
    Wq = np.ascontiguousarray(Wq, dtype=np.float32)
    bq = np.ascontiguousarray(bq, dtype=np.float32)
    Wk = np.ascontiguousarray(Wk, dtype=np.float32)
    Wv = np.ascontiguousarray(Wv, dtype=np.float32)
    bv32 = (32.0 * np.asarray(bv)).astype(np.float32)

    if "nc" not in _NC_CACHE:
        _NC_CACHE["nc"] = _build_graph()
    nc = _NC_CACHE["nc"]

    in_maps = []
    for i in range(NCORES):
        in_maps.append({
            "P": np.ascontiguousarray(P[:, i * NLOC:(i + 1) * NLOC, :]),
            "M": np.ascontiguousarray(M[:, i * NLOC * mV:(i + 1) * NLOC * mV, :]),
            "maskf": np.ascontiguousarray(maskrow[i * NLOC:(i + 1) * NLOC, :]),
            "Wq": Wq, "bq": bq, "Wk": Wk, "Wv": Wv, "bv32": bv32,
        })
    res = run_bass_kernel_spmd(nc, in_maps, core_ids=list(range(NCORES)))
    outs = res.results
    out = np.concatenate([outs[i]["out"] for i in range(NCORES)], axis=1)
    att = np.concatenate([outs[i]["att"] for i in range(NCORES)], axis=0)
    return out.astype(np.float32), att.astype(np.float32)


# revision 28
# speedup vs baseline: 1.3949x; 1.3949x over previous
"""Sparse-attention Trainium2 kernel (8 NeuronCores, data-parallel over N).

Math (per batch element n, derived from the reference):
    Q  = Pn @ Wq.T + bq                      [T, C]
    scores = Q @ K.T  with K = Mn @ Wk.T + bk
           = (Q @ Wk) @ Mn.T + (Q . bk)      -- bk term is constant over the
                                                softmax group (mV axis) -> dropped
    Q' = Q @ Wk                              [T, C]
    sm = scores + additive_mask              (masked lanes == -1e15 exactly in f32)
    attn = softmax over mV groups            (per-t row max as the exp shift --
                                              safe: observed per-row group-max gap
                                              << 87, so no under/overflow)
    out = attn @ V,  V = Mn @ Wv.T + bv
        = (attn @ Mn) @ Wv.T + 32*bv         -- since sum_s attn[t, s] == 32 exactly
    att = mean over (t, tm) of attn          [mV]

Layouts on chip (partition dim first):
    PT   [c', (cc, t)]      via PE transpose of P natural
    QT   [c, (cc, t)]       = WqT.T @ PT    (+bq via ACT bias)
    Q'T  [c', (cc, t)]      = Wk_nat.T @ QT
    Mn   [s, (sc, c)]       natural DMA load (used as lhsT for Z)
    MnT  [c', (cc, s)]      via PE transpose (moving operand for scores)
    scores PSUM [t, s-slice] = Q'T.T @ MnT, + mask via K=1 matmul;
    exp on ACT with bias=-rowmax (fused PSUM->SBUF)
    attn [t, (tc, s)]       normalized on DVE, transposed to attnT [s, (sc, t)]
    ZT   [c', (cc, t)]      = Mn.T @ attnT
    out  [t, (tc, c)]       = ZT.T @ WvT + 32*bv
"""

import sys

for _p in ("/opt/trn_rl_repo",):
    if _p not in sys.path:
        sys.path.insert(0, _p)

from contextlib import ExitStack

import numpy as np

import concourse.bass as bass
from concourse import bacc
import concourse.mybir as mybir
import concourse.tile as tile
from concourse.bass_utils import run_bass_kernel_spmd
from concourse.masks import make_identity

T, N, C = 256, 64, 256
mV, mT = 64, 32
S = mV * mT          # 2048
NCORES = 8
NLOC = N // NCORES   # 8 batch elements per core
TC = T // 128        # 2 t-chunks
CC = C // 128        # 2 c-chunks
SC = S // 128        # 16 s-chunks
SL = S // 512        # 4 s-slices (fp32 moving max)
SG = SC // 4         # 4 transpose groups of 4 chunks

F32 = mybir.dt.float32
F32R = mybir.dt.float32r
BF16 = mybir.dt.bfloat16
AX = mybir.AxisListType
AF = mybir.ActivationFunctionType


def r(ap):
    """Bitcast an AP to float32r for full-rate fp32 matmuls."""
    return ap.bitcast(F32R)


def _build_graph():
    nc = bacc.Bacc()

    P_d = nc.declare_dram_parameter("P", [T, NLOC, C], F32, isOutput=False)
    M_d = nc.declare_dram_parameter("M", [mT, NLOC * mV, C], F32, isOutput=False)
    mk_d = nc.declare_dram_parameter("maskf", [NLOC, S], F32, isOutput=False)
    Wq_d = nc.declare_dram_parameter("Wq", [C, C], F32, isOutput=False)
    bq_d = nc.declare_dram_parameter("bq", [C], F32, isOutput=False)
    Wk_d = nc.declare_dram_parameter("Wk", [C, C], F32, isOutput=False)
    Wv_d = nc.declare_dram_parameter("Wv", [C, C], F32, isOutput=False)
    bv32_d = nc.declare_dram_parameter("bv32", [C], F32, isOutput=False)
    out_d = nc.declare_dram_parameter("out", [T, NLOC, C], F32, isOutput=True)
    att_d = nc.declare_dram_parameter("att", [NLOC, mV], F32, isOutput=True)

    with tile.TileContext(nc) as tc, ExitStack() as ctx:
        const = ctx.enter_context(tc.tile_pool(name="const", bufs=1))
        mn_pool = ctx.enter_context(tc.tile_pool(name="mn", bufs=2))
        mnt_pool = ctx.enter_context(tc.tile_pool(name="mnt", bufs=2))
        attn_pool = ctx.enter_context(tc.tile_pool(name="attn", bufs=2))
        attnt_pool = ctx.enter_context(tc.tile_pool(name="attnt", bufs=2))
        work = ctx.enter_context(tc.tile_pool(name="work", bufs=2))
        outp = ctx.enter_context(tc.tile_pool(name="outp", bufs=3))
        ps_big = ctx.enter_context(tc.tile_pool(name="psbig", bufs=4, space="PSUM"))
        ps_tr = ctx.enter_context(tc.tile_pool(name="pstr", bufs=2, space="PSUM"))
        ps_sm = ctx.enter_context(tc.tile_pool(name="pssm", bufs=2, space="PSUM"))

        # ---- constants ----
        ident = const.tile([128, 128], F32)
        make_identity(nc, ident)
        identb = const.tile([128, 128], BF16)
        make_identity(nc, identb)
        onecol_b = const.tile([128, 1], BF16)
        nc.vector.memset(onecol_b, 1.0)
        ones_raw = const.tile([1, 128], F32)
        nc.vector.memset(ones_raw, 1.0)
        ones1 = const.tile([1, 128], F32)
        nc.scalar.copy(r(ones1), ones_raw)

        # weights: single batched DMA each
        wq_nat = const.tile([128, CC, C], F32)
        wv_nat = const.tile([128, CC, C], F32)
        wk_sb = const.tile([128, CC, C], F32)   # Wk natural [co, (coc, ci)] used as-is
        nc.sync.dma_start(out=wq_nat, in_=Wq_d[:, :].rearrange("(cc p) ci -> p cc ci", p=128))
        nc.sync.dma_start(out=wv_nat, in_=Wv_d[:, :].rearrange("(cc p) ci -> p cc ci", p=128))
        nc.sync.dma_start(out=r(wk_sb), in_=r(Wk_d[:, :].rearrange("(cc p) ci -> p cc ci", p=128)))
        wqt = const.tile([128, CC, C], F32)     # [ci_loc, cic, co]
        wvt = const.tile([128, CC, C], F32)
        for cic in range(CC):
            pst = ps_tr.tile([128, 512], F32, tag="pstr")
            nc.tensor.transpose(pst[:, 0:128], wq_nat[:, 0, cic * 128:(cic + 1) * 128], ident)
            nc.tensor.transpose(pst[:, 128:256], wq_nat[:, 1, cic * 128:(cic + 1) * 128], ident)
            nc.tensor.transpose(pst[:, 256:384], wv_nat[:, 0, cic * 128:(cic + 1) * 128], ident)
            nc.tensor.transpose(pst[:, 384:512], wv_nat[:, 1, cic * 128:(cic + 1) * 128], ident)
            nc.scalar.copy(r(wqt[:, cic, :]), pst[:, 0:256])
            nc.scalar.copy(r(wvt[:, cic, :]), pst[:, 256:512])

        bq_sb = const.tile([128, CC], F32)
        for cc in range(CC):
            nc.sync.dma_start(
                out=bq_sb[:, cc:cc + 1],
                in_=bq_d[cc * 128:(cc + 1) * 128].unsqueeze(1),
            )
        bv32_sb = const.tile([128, C], F32)
        nc.sync.dma_start(out=bv32_sb, in_=bv32_d[:].partition_broadcast(128))

        # ---- software-pipelined per-batch-element stages ----
        # prep(n):  P/M loads, PT/QT/Q'T, MnT transposes  (PE/ACT/DMA)
        # soft(n):  scores, group-max softmax, normalize, attnT  (PE/DVE/ACT)
        # tail(n):  att matmuls, ZT, out  (PE)
        # Emission order interleaves prep(i+2) | soft(i+1)+tail-of-PE work so the
        # PE stream never waits long on the DVE softmax chain.
        state = {}

        def prep(n):
            p_nat = work.tile([128, TC, C], F32, tag="pnat")
            nc.sync.dma_start(
                out=p_nat,
                in_=P_d[:, n, :].rearrange("(tc p) c -> p tc c", p=128),
            )
            pt = work.tile([128, CC, T], F32, tag="pt")
            pstp = ps_tr.tile([128, 512], F32, tag="pstr")
            for tc_i in range(TC):
                for cc in range(CC):
                    nc.tensor.transpose(
                        pstp[:, (cc * TC + tc_i) * 128:(cc * TC + tc_i + 1) * 128],
                        p_nat[:, tc_i, cc * 128:(cc + 1) * 128], ident,
                    )
            for cc in range(CC):
                nc.scalar.copy(r(pt[:, cc, :]), pstp[:, cc * 256:(cc + 1) * 256])

            qt = work.tile([128, CC, T], F32, tag="qt")
            for cc in range(CC):
                psq = ps_sm.tile([128, T], F32, tag="pssm")
                for k in range(CC):
                    nc.tensor.matmul(
                        psq, r(wqt[:, k, cc * 128:(cc + 1) * 128]), r(pt[:, k, :]),
                        start=(k == 0), stop=(k == CC - 1),
                    )
                nc.scalar.add(r(qt[:, cc, :]), psq, add=bq_sb[:, cc:cc + 1])

            qpt = work.tile([128, CC, T], F32, tag="qpt")
            for cc in range(CC):
                psq = ps_sm.tile([128, T], F32, tag="pssm")
                for k in range(CC):
                    nc.tensor.matmul(
                        psq, r(wk_sb[:, k, cc * 128:(cc + 1) * 128]), r(qt[:, k, :]),
                        start=(k == 0), stop=(k == CC - 1),
                    )
                nc.scalar.copy(r(qpt[:, cc, :]), psq)

            # mn partitions are tm-major: p = tm*4 + vl (matches M's DRAM
            # order, giving 32x4KB DMA descriptors instead of 128x1KB)
            mn = mn_pool.tile([128, SC, C], F32, tag="mn")
            for sc in range(SC):
                msrc = M_d[:, n * mV + sc * 4:n * mV + (sc + 1) * 4, :]
                nc.sync.dma_start(out=r(mn[:, sc, :]), in_=r(msrc))
            mnt = mnt_pool.tile([128, CC, S], F32, tag="mnt")
            for cc in range(CC):
                for g in range(SG):
                    pst = ps_tr.tile([128, 512], F32, tag="pstr")
                    for j in range(4):
                        nc.tensor.transpose(
                            pst[:, j * 128:(j + 1) * 128],
                            mn[:, g * 4 + j, cc * 128:(cc + 1) * 128], ident,
                        )
                    nc.scalar.copy(r(mnt[:, cc, g * 512:(g + 1) * 512]), pst)

            mrow = work.tile([1, S], F32, tag="mrow")
            nc.sync.dma_start(out=r(mrow), in_=r(mk_d[n:n + 1, :]))
            state[n] = dict(qpt=qpt, mn=mn, mnt=mnt, mrow=mrow)

        def soft(n):
            st = state[n]
            qpt, mnt, mrow = st["qpt"], st["mnt"], st["mrow"]
            attn_sb = attn_pool.tile([128, TC, S], BF16, tag="attn")
            attnt = attnt_pool.tile([128, SC, T], F32, tag="attnt")
            gv = work.tile([128, TC, mV], BF16, tag="gv")
            for tc_i in range(TC):
                slices = []
                for k in range(CC):
                    for sl in range(SL):
                        if k == 0:
                            pss = ps_big.tile([128, 512], F32, tag="psbig")
                            slices.append(pss)
                        nc.tensor.matmul(
                            slices[sl],
                            r(qpt[:, k, tc_i * 128:(tc_i + 1) * 128]),
                            r(mnt[:, k, sl * 512:(sl + 1) * 512]),
                            start=(k == 0), stop=False,
                        )
                pm = work.tile([128, SL, mT], F32, tag="pm")
                for sl in range(SL):
                    nc.tensor.matmul(
                        slices[sl], r(ones1), r(mrow[:, sl * 512:(sl + 1) * 512]),
                        start=False, stop=True,
                    )
                    nc.vector.reduce_max(
                        out=pm[:, sl, :],
                        in_=slices[sl][:, :]
                        .rearrange("p (scl tm vl) -> p tm scl vl", tm=mT, vl=4),
                        axis=AX.XY,
                    )
                mx = work.tile([128, mT], F32, tag="mx")
                nc.vector.reduce_max(
                    out=mx, in_=pm.rearrange("p sl tm -> p tm sl"), axis=AX.X,
                )
                sc_f = work.tile([128, S], F32, tag="scf")
                for sl in range(SL):
                    nc.vector.tensor_sub(
                        sc_f[:, sl * 512:(sl + 1) * 512]
                        .rearrange("p (scl tm vl) -> p scl tm vl", tm=mT, vl=4),
                        slices[sl][:, :]
                        .rearrange("p (scl tm vl) -> p scl tm vl", tm=mT, vl=4),
                        mx.unsqueeze(1).unsqueeze(3).broadcast_to([128, 4, mT, 4]),
                    )
                    nc.scalar.activation(
                        attn_sb[:, tc_i, sl * 512:(sl + 1) * 512],
                        sc_f[:, sl * 512:(sl + 1) * 512], AF.Exp,
                        bias=0.0, scale=1.0,
                    )
                # denominator: contiguous binary-tree sum over the mV axis
                half = S // 2
                dtree = work.tile([128, half], BF16, tag="dtree")
                nc.vector.tensor_add(
                    dtree[:, 0:half],
                    attn_sb[:, tc_i, 0:half], attn_sb[:, tc_i, half:S],
                )
                w = half // 2
                while w >= 128:
                    nc.vector.tensor_add(
                        dtree[:, 0:w], dtree[:, 0:w], dtree[:, w:2 * w],
                    )
                    w //= 2
                denom = work.tile([128, mT], F32, tag="denom")
                nc.vector.reduce_sum(
                    out=denom,
                    in_=dtree[:, 0:128].rearrange("p (tm vl) -> p tm vl", vl=4),
                    axis=AX.X,
                )
                recip = work.tile([128, mT], BF16, tag="recip")
                with nc.allow_low_precision(reason="attn weights <= 1; bf16 ok"):
                    nc.vector.reciprocal(recip, denom)
                recip_exp = work.tile([128, 128], BF16, tag="recipexp")
                nc.vector.tensor_copy(
                    recip_exp.rearrange("p (tm vl) -> p tm vl", vl=4),
                    recip.unsqueeze(2).broadcast_to([128, mT, 4]),
                )
                nc.vector.tensor_mul(
                    attn_sb[:, tc_i, :].rearrange("p (sc f) -> p sc f", f=128),
                    attn_sb[:, tc_i, :].rearrange("p (sc f) -> p sc f", f=128),
                    recip_exp.unsqueeze(1).broadcast_to([128, SC, 128]),
                )
                # transpose attn -> attnT [s, (sc, t)] (batches of 8 blocks)
                for g in range(2):
                    pstb = ps_big.tile([128, 8, 128], BF16, tag="psbig")
                    for j in range(8):
                        nc.tensor.transpose(
                            pstb[:, j, :],
                            attn_sb[:, tc_i, (g * 8 + j) * 128:(g * 8 + j + 1) * 128],
                            identb,
                        )
                    nc.scalar.copy(
                        r(attnt[:, g * 8:(g + 1) * 8, tc_i * 128:(tc_i + 1) * 128]),
                        pstb,
                    )
                # per-(t, v) sums over tm for att (off the critical path)
                with nc.allow_low_precision(reason="att partials; bf16 ok"):
                    nc.vector.reduce_sum(
                        out=gv[:, tc_i, :],
                        in_=attn_sb[:, tc_i, :]
                        .rearrange("p (sc tm vl) -> p sc vl tm", tm=mT, vl=4),
                        axis=AX.X,
                    )
            st["attnt"] = attnt
            st["gv"] = gv

        def tail(n):
            st = state.pop(n)
            mn, attnt, gv = st["mn"], st["attnt"], st["gv"]
            # att[v] = (1/(T*mT)) * sum_t gv[t, v]: ones-matmul over partitions
            psa = ps_sm.tile([mV, 1], F32, tag="pssm")
            for tc_i in range(TC):
                nc.tensor.matmul(
                    psa, gv[:, tc_i, :], onecol_b,
                    start=(tc_i == 0), stop=(tc_i == TC - 1),
                )
            att_fin = outp.tile([mV, 1], F32, tag="attfin")
            nc.scalar.mul(att_fin, psa, 1.0 / (T * mT))
            nc.sync.dma_start(out=att_d[n, :].unsqueeze(1), in_=att_fin)

            zt = work.tile([128, CC, T], F32, tag="zt")
            for cc in range(CC):
                psz = ps_sm.tile([128, T], F32, tag="pssm")
                for sc in range(SC):
                    nc.tensor.matmul(
                        psz, r(mn[:, sc, cc * 128:(cc + 1) * 128]), r(attnt[:, sc, :]),
                        start=(sc == 0), stop=(sc == SC - 1),
                    )
                nc.scalar.copy(r(zt[:, cc, :]), psz)

            o_sb = outp.tile([128, TC, C], F32, tag="osb")
            for tc_i in range(TC):
                pso = ps_sm.tile([128, C], F32, tag="pssm")
                for k in range(CC):
                    nc.tensor.matmul(
                        pso, r(zt[:, k, tc_i * 128:(tc_i + 1) * 128]), r(wvt[:, k, :]),
                        start=(k == 0), stop=(k == CC - 1),
                    )
                nc.vector.tensor_add(o_sb[:, tc_i, :], pso, bv32_sb)
            nc.sync.dma_start(
                out=out_d[:, n, :].rearrange("(tc p) c -> p tc c", p=128), in_=o_sb,
            )

        # staggered emission: prep two ahead, softmax one ahead, tail current
        prep(0)
        prep(1)
        soft(0)
        for i in range(NLOC):
            if i + 2 < NLOC:
                prep(i + 2)
            if i + 1 < NLOC:
                soft(i + 1)
            tail(i)

    nc.finalize()
    return nc


_NC_CACHE = {}


def kernel(P, M, mask, Wq, bq, Wk, bk, Wv, bv):
    P = np.ascontiguousarray(P, dtype=np.float32)
    M = np.ascontiguousarray(M, dtype=np.float32)
    maskf = np.where(np.asarray(mask), np.float32(0.0), np.float32(-1e15)).astype(np.float32)
    # permuted on-chip s layout: s = sc*128 + tm*4 + vl  (v = sc*4 + vl)
    maskrow = np.ascontiguousarray(
        np.broadcast_to(
            maskf.reshape(N, SC, 1, 4), (N, SC, mT, 4)
        ).reshape(N, S)
    )
    Wq = np.ascontiguousarray(Wq, dtype=np.float32)
    bq = np.ascontiguousarray(bq, dtype=np.float32)
    Wk = np.ascontiguousarray(Wk, dtype=np.float32)
    Wv = np.ascontiguousarray(Wv, dtype=np.float32)
    bv32 = (32.0 * np.asarray(bv)).astype(np.float32)

    if "nc" not in _NC_CACHE:
        _NC_CACHE["nc"] = _build_graph()
    nc = _NC_CACHE["nc"]

    in_maps = []
    for i in range(NCORES):
        in_maps.append({
            "P": np.ascontiguousarray(P[:, i * NLOC:(i + 1) * NLOC, :]),
            "M": np.ascontiguousarray(M[:, i * NLOC * mV:(i + 1) * NLOC * mV, :]),
            "maskf": np.ascontiguousarray(maskrow[i * NLOC:(i + 1) * NLOC, :]),
            "Wq": Wq, "bq": bq, "Wk": Wk, "Wv": Wv, "bv32": bv32,
        })
    res = run_bass_kernel_spmd(nc, in_maps, core_ids=list(range(NCORES)))
    outs = res.results
    out = np.concatenate([outs[i]["out"] for i in range(NCORES)], axis=1)
    att = np.concatenate([outs[i]["att"] for i in range(NCORES)], axis=0)
    return out.astype(np.float32), att.astype(np.float32)


# revision 31
# speedup vs baseline: 1.5748x; 1.1290x over previous
"""Sparse-attention Trainium2 kernel (8 NeuronCores, data-parallel over N).

Math (per batch element n, derived from the reference):
    Q  = Pn @ Wq.T + bq                      [T, C]
    scores = Q @ K.T  with K = Mn @ Wk.T + bk
           = (Q @ Wk) @ Mn.T + (Q . bk)      -- bk term is constant over the
                                                softmax group (mV axis) -> dropped
    Q' = Q @ Wk                              [T, C]
    sm = scores + additive_mask              (masked lanes == -1e15 exactly in f32)
    attn = softmax over mV groups            (per-t row max as the exp shift --
                                              safe: observed per-row group-max gap
                                              << 87, so no under/overflow)
    out = attn @ V,  V = Mn @ Wv.T + bv
        = (attn @ Mn) @ Wv.T + 32*bv         -- since sum_s attn[t, s] == 32 exactly
    att = mean over (t, tm) of attn          [mV]

Layouts on chip (partition dim first):
    PT   [c', (cc, t)]      via PE transpose of P natural
    QT   [c, (cc, t)]       = WqT.T @ PT    (+bq via ACT bias)
    Q'T  [c', (cc, t)]      = Wk_nat.T @ QT
    Mn   [s, (sc, c)]       natural DMA load (used as lhsT for Z)
    MnT  [c', (cc, s)]      via PE transpose (moving operand for scores)
    scores PSUM [t, s-slice] = Q'T.T @ MnT, + mask via K=1 matmul;
    exp on ACT with bias=-rowmax (fused PSUM->SBUF)
    attn [t, (tc, s)]       normalized on DVE, transposed to attnT [s, (sc, t)]
    ZT   [c', (cc, t)]      = Mn.T @ attnT
    out  [t, (tc, c)]       = ZT.T @ WvT + 32*bv
"""

import sys

for _p in ("/opt/trn_rl_repo",):
    if _p not in sys.path:
        sys.path.insert(0, _p)

from contextlib import ExitStack

import numpy as np

import concourse.bass as bass
from concourse import bacc
import concourse.mybir as mybir
import concourse.tile as tile
from concourse.bass_utils import run_bass_kernel_spmd
from concourse.masks import make_identity

T, N, C = 256, 64, 256
mV, mT = 64, 32
S = mV * mT          # 2048
NCORES = 8
NLOC = N // NCORES   # 8 batch elements per core
TC = T // 128        # 2 t-chunks
CC = C // 128        # 2 c-chunks
SC = S // 128        # 16 s-chunks
SL = S // 512        # 4 s-slices (fp32 moving max)
SG = SC // 4         # 4 transpose groups of 4 chunks

F32 = mybir.dt.float32
F32R = mybir.dt.float32r
BF16 = mybir.dt.bfloat16
AX = mybir.AxisListType
AF = mybir.ActivationFunctionType


def r(ap):
    """Bitcast an AP to float32r for full-rate fp32 matmuls."""
    return ap.bitcast(F32R)


def _build_graph():
    nc = bacc.Bacc()

    P_d = nc.declare_dram_parameter("P", [T, NLOC, C], F32, isOutput=False)
    M_d = nc.declare_dram_parameter("M", [mT, NLOC * mV, C], F32, isOutput=False)
    mk_d = nc.declare_dram_parameter("maskf", [NLOC, S], F32, isOutput=False)
    Wq_d = nc.declare_dram_parameter("Wq", [C, C], F32, isOutput=False)
    bq_d = nc.declare_dram_parameter("bq", [C], F32, isOutput=False)
    Wk_d = nc.declare_dram_parameter("Wk", [C, C], F32, isOutput=False)
    Wv_d = nc.declare_dram_parameter("Wv", [C, C], F32, isOutput=False)
    bv32_d = nc.declare_dram_parameter("bv32", [C], F32, isOutput=False)
    out_d = nc.declare_dram_parameter("out", [T, NLOC, C], F32, isOutput=True)
    att_d = nc.declare_dram_parameter("att", [NLOC, mV], F32, isOutput=True)

    with tile.TileContext(nc) as tc, ExitStack() as ctx:
        const = ctx.enter_context(tc.tile_pool(name="const", bufs=1))
        mn_pool = ctx.enter_context(tc.tile_pool(name="mn", bufs=2))
        mnt_pool = ctx.enter_context(tc.tile_pool(name="mnt", bufs=2))
        attn_pool = ctx.enter_context(tc.tile_pool(name="attn", bufs=2))
        attnt_pool = ctx.enter_context(tc.tile_pool(name="attnt", bufs=2))
        work = ctx.enter_context(tc.tile_pool(name="work", bufs=2))
        outp = ctx.enter_context(tc.tile_pool(name="outp", bufs=3))
        ps_big = ctx.enter_context(tc.tile_pool(name="psbig", bufs=4, space="PSUM"))
        ps_tr = ctx.enter_context(tc.tile_pool(name="pstr", bufs=2, space="PSUM"))
        ps_sm = ctx.enter_context(tc.tile_pool(name="pssm", bufs=2, space="PSUM"))

        # ---- constants ----
        ident = const.tile([128, 128], F32)
        make_identity(nc, ident)
        identb = const.tile([128, 128], BF16)
        make_identity(nc, identb)
        onecol_b = const.tile([128, 1], BF16)
        nc.vector.memset(onecol_b, 1.0)
        ones_raw = const.tile([1, 128], F32)
        nc.vector.memset(ones_raw, 1.0)
        ones1 = const.tile([1, 128], F32)
        nc.scalar.copy(r(ones1), ones_raw)

        # weights: single batched DMA each
        wq_nat = const.tile([128, CC, C], F32)
        wv_nat = const.tile([128, CC, C], F32)
        wk_sb = const.tile([128, CC, C], F32)   # Wk natural [co, (coc, ci)] used as-is
        nc.sync.dma_start(out=wq_nat, in_=Wq_d[:, :].rearrange("(cc p) ci -> p cc ci", p=128))
        nc.sync.dma_start(out=wv_nat, in_=Wv_d[:, :].rearrange("(cc p) ci -> p cc ci", p=128))
        nc.sync.dma_start(out=r(wk_sb), in_=r(Wk_d[:, :].rearrange("(cc p) ci -> p cc ci", p=128)))
        wqt = const.tile([128, CC, C], F32)     # [ci_loc, cic, co]
        wvt = const.tile([128, CC, C], F32)
        for cic in range(CC):
            pst = ps_tr.tile([128, 512], F32, tag="pstr")
            nc.tensor.transpose(pst[:, 0:128], wq_nat[:, 0, cic * 128:(cic + 1) * 128], ident)
            nc.tensor.transpose(pst[:, 128:256], wq_nat[:, 1, cic * 128:(cic + 1) * 128], ident)
            nc.tensor.transpose(pst[:, 256:384], wv_nat[:, 0, cic * 128:(cic + 1) * 128], ident)
            nc.tensor.transpose(pst[:, 384:512], wv_nat[:, 1, cic * 128:(cic + 1) * 128], ident)
            nc.scalar.copy(r(wqt[:, cic, :]), pst[:, 0:256])
            nc.scalar.copy(r(wvt[:, cic, :]), pst[:, 256:512])

        bq_sb = const.tile([128, CC], F32)
        for cc in range(CC):
            nc.sync.dma_start(
                out=bq_sb[:, cc:cc + 1],
                in_=bq_d[cc * 128:(cc + 1) * 128].unsqueeze(1),
            )
        bv32_sb = const.tile([128, C], F32)
        nc.sync.dma_start(out=bv32_sb, in_=bv32_d[:].partition_broadcast(128))

        # ---- software-pipelined per-batch-element stages ----
        # prep(n):  P/M loads, PT/QT/Q'T, MnT transposes  (PE/ACT/DMA)
        # soft(n):  scores, group-max softmax, normalize, attnT  (PE/DVE/ACT)
        # tail(n):  att matmuls, ZT, out  (PE)
        # Emission order interleaves prep(i+2) | soft(i+1)+tail-of-PE work so the
        # PE stream never waits long on the DVE softmax chain.
        state = {}

        def prep(n):
            p_nat = work.tile([128, TC, C], F32, tag="pnat")
            nc.sync.dma_start(
                out=p_nat,
                in_=P_d[:, n, :].rearrange("(tc p) c -> p tc c", p=128),
            )
            pt = work.tile([128, CC, T], F32, tag="pt")
            pstp = ps_tr.tile([128, 512], F32, tag="pstr")
            for tc_i in range(TC):
                for cc in range(CC):
                    nc.tensor.transpose(
                        pstp[:, (cc * TC + tc_i) * 128:(cc * TC + tc_i + 1) * 128],
                        p_nat[:, tc_i, cc * 128:(cc + 1) * 128], ident,
                    )
            for cc in range(CC):
                nc.scalar.copy(r(pt[:, cc, :]), pstp[:, cc * 256:(cc + 1) * 256])

            qt = work.tile([128, CC, T], F32, tag="qt")
            for cc in range(CC):
                psq = ps_sm.tile([128, T], F32, tag="pssm")
                for k in range(CC):
                    nc.tensor.matmul(
                        psq, r(wqt[:, k, cc * 128:(cc + 1) * 128]), r(pt[:, k, :]),
                        start=(k == 0), stop=(k == CC - 1),
                    )
                nc.scalar.add(r(qt[:, cc, :]), psq, add=bq_sb[:, cc:cc + 1])

            qpt = work.tile([128, CC, T], F32, tag="qpt")
            for cc in range(CC):
                psq = ps_sm.tile([128, T], F32, tag="pssm")
                for k in range(CC):
                    nc.tensor.matmul(
                        psq, r(wk_sb[:, k, cc * 128:(cc + 1) * 128]), r(qt[:, k, :]),
                        start=(k == 0), stop=(k == CC - 1),
                    )
                nc.scalar.copy(r(qpt[:, cc, :]), psq)

            # mn partitions are tm-major: p = tm*4 + vl (matches M's DRAM
            # order, giving 32x4KB DMA descriptors instead of 128x1KB)
            mn = mn_pool.tile([128, SC, C], F32, tag="mn")
            for sc in range(SC):
                msrc = M_d[:, n * mV + sc * 4:n * mV + (sc + 1) * 4, :]
                nc.sync.dma_start(out=r(mn[:, sc, :]), in_=r(msrc))
            mnt = mnt_pool.tile([128, CC, S], F32, tag="mnt")
            for cc in range(CC):
                for g in range(SG):
                    pst = ps_tr.tile([128, 512], F32, tag="pstr")
                    for j in range(4):
                        nc.tensor.transpose(
                            pst[:, j * 128:(j + 1) * 128],
                            mn[:, g * 4 + j, cc * 128:(cc + 1) * 128], ident,
                        )
                    nc.scalar.copy(r(mnt[:, cc, g * 512:(g + 1) * 512]), pst)

            mrow = work.tile([1, S], F32, tag="mrow")
            nc.sync.dma_start(out=r(mrow), in_=r(mk_d[n:n + 1, :]))
            state[n] = dict(qpt=qpt, mn=mn, mnt=mnt, mrow=mrow)

        def soft_scores(n, tc_i):
            st = state[n]
            qpt, mnt, mrow = st["qpt"], st["mnt"], st["mrow"]
            if tc_i == 0:
                attn_sb = attn_pool.tile([128, TC, S], BF16, tag="attn")
                attnt = attnt_pool.tile([128, SC, T], F32, tag="attnt")
                gv = work.tile([128, TC, mV], BF16, tag="gv")
                st["attn_sb"], st["attnt"], st["gv"] = attn_sb, attnt, gv
            slices = []
            for k in range(CC):
                for sl in range(SL):
                    if k == 0:
                        pss = ps_big.tile([128, 512], F32, tag="psbig")
                        slices.append(pss)
                    nc.tensor.matmul(
                        slices[sl],
                        r(qpt[:, k, tc_i * 128:(tc_i + 1) * 128]),
                        r(mnt[:, k, sl * 512:(sl + 1) * 512]),
                        start=(k == 0), stop=False,
                    )
            pm = work.tile([128, SL, mT], F32, tag="pm")
            for sl in range(SL):
                nc.tensor.matmul(
                    slices[sl], r(ones1), r(mrow[:, sl * 512:(sl + 1) * 512]),
                    start=False, stop=True,
                )
                nc.vector.reduce_max(
                    out=pm[:, sl, :],
                    in_=slices[sl][:, :]
                    .rearrange("p (scl tm vl) -> p tm scl vl", tm=mT, vl=4),
                    axis=AX.XY,
                )
            st[("slices", tc_i)] = slices
            st[("pm", tc_i)] = pm

        def soft_main(n, tc_i):
            st = state[n]
            attn_sb = st["attn_sb"]
            slices = st.pop(("slices", tc_i))
            pm = st.pop(("pm", tc_i))
            mx = work.tile([128, mT], F32, tag="mx")
            nc.vector.reduce_max(
                out=mx, in_=pm.rearrange("p sl tm -> p tm sl"), axis=AX.X,
            )
            mx_exp = work.tile([128, 128], F32, tag="mxe")
            nc.vector.tensor_copy(
                mx_exp.rearrange("p (tm vl) -> p tm vl", vl=4),
                mx.unsqueeze(2).broadcast_to([128, mT, 4]),
            )
            sc_f = work.tile([128, S], F32, tag="scf")
            for sl in range(SL):
                nc.vector.tensor_sub(
                    sc_f[:, sl * 512:(sl + 1) * 512]
                    .rearrange("p (scl f) -> p scl f", f=128),
                    slices[sl][:, :].rearrange("p (scl f) -> p scl f", f=128),
                    mx_exp.unsqueeze(1).broadcast_to([128, 4, 128]),
                )
                nc.scalar.activation(
                    attn_sb[:, tc_i, sl * 512:(sl + 1) * 512],
                    sc_f[:, sl * 512:(sl + 1) * 512], AF.Exp,
                    bias=0.0, scale=1.0,
                )
            half = S // 2
            dtree = work.tile([128, half], BF16, tag="dtree")
            nc.vector.tensor_add(
                dtree[:, 0:half],
                attn_sb[:, tc_i, 0:half], attn_sb[:, tc_i, half:S],
            )
            w = half // 2
            while w >= 128:
                nc.vector.tensor_add(
                    dtree[:, 0:w], dtree[:, 0:w], dtree[:, w:2 * w],
                )
                w //= 2
            denom = work.tile([128, mT], F32, tag="denom")
            nc.vector.reduce_sum(
                out=denom,
                in_=dtree[:, 0:128].rearrange("p (tm vl) -> p tm vl", vl=4),
                axis=AX.X,
            )
            recip = work.tile([128, mT], BF16, tag="recip")
            with nc.allow_low_precision(reason="attn weights <= 1; bf16 ok"):
                nc.vector.reciprocal(recip, denom)
            recip_exp = work.tile([128, 128], BF16, tag="recipexp")
            nc.vector.tensor_copy(
                recip_exp.rearrange("p (tm vl) -> p tm vl", vl=4),
                recip.unsqueeze(2).broadcast_to([128, mT, 4]),
            )
            nc.vector.tensor_mul(
                attn_sb[:, tc_i, :].rearrange("p (sc f) -> p sc f", f=128),
                attn_sb[:, tc_i, :].rearrange("p (sc f) -> p sc f", f=128),
                recip_exp.unsqueeze(1).broadcast_to([128, SC, 128]),
            )

        def soft_tr(n, tc_i):
            st = state[n]
            attn_sb, attnt = st["attn_sb"], st["attnt"]
            for g in range(2):
                pstb = ps_big.tile([128, 8, 128], BF16, tag="psbig")
                for j in range(8):
                    nc.tensor.transpose(
                        pstb[:, j, :],
                        attn_sb[:, tc_i, (g * 8 + j) * 128:(g * 8 + j + 1) * 128],
                        identb,
                    )
                nc.scalar.copy(
                    r(attnt[:, g * 8:(g + 1) * 8, tc_i * 128:(tc_i + 1) * 128]),
                    pstb,
                )

        def soft_gv(n):
            st = state[n]
            attn_sb, gv = st["attn_sb"], st["gv"]
            for tc_i in range(TC):
                with nc.allow_low_precision(reason="att partials; bf16 ok"):
                    nc.vector.reduce_sum(
                        out=gv[:, tc_i, :],
                        in_=attn_sb[:, tc_i, :]
                        .rearrange("p (sc tm vl) -> p sc vl tm", tm=mT, vl=4),
                        axis=AX.X,
                    )

        def tail(n):
            st = state.pop(n)
            mn, attnt, gv = st["mn"], st["attnt"], st["gv"]
            # att[v] = (1/(T*mT)) * sum_t gv[t, v]: ones-matmul over partitions
            psa = ps_sm.tile([mV, 1], F32, tag="pssm")
            for tc_i in range(TC):
                nc.tensor.matmul(
                    psa, gv[:, tc_i, :], onecol_b,
                    start=(tc_i == 0), stop=(tc_i == TC - 1),
                )
            att_fin = outp.tile([mV, 1], F32, tag="attfin")
            nc.scalar.mul(att_fin, psa, 1.0 / (T * mT))
            nc.sync.dma_start(out=att_d[n, :].unsqueeze(1), in_=att_fin)

            zt = work.tile([128, CC, T], F32, tag="zt")
            for cc in range(CC):
                psz = ps_sm.tile([128, T], F32, tag="pssm")
                for sc in range(SC):
                    nc.tensor.matmul(
                        psz, r(mn[:, sc, cc * 128:(cc + 1) * 128]), r(attnt[:, sc, :]),
                        start=(sc == 0), stop=(sc == SC - 1),
                    )
                nc.scalar.copy(r(zt[:, cc, :]), psz)

            o_sb = outp.tile([128, TC, C], F32, tag="osb")
            for tc_i in range(TC):
                pso = ps_sm.tile([128, C], F32, tag="pssm")
                for k in range(CC):
                    nc.tensor.matmul(
                        pso, r(zt[:, k, tc_i * 128:(tc_i + 1) * 128]), r(wvt[:, k, :]),
                        start=(k == 0), stop=(k == CC - 1),
                    )
                nc.vector.tensor_add(o_sb[:, tc_i, :], pso, bv32_sb)
            nc.sync.dma_start(
                out=out_d[:, n, :].rearrange("(tc p) c -> p tc c", p=128), in_=o_sb,
            )

        # staggered emission: PE filler work slotted inside each DVE block
        prep(0)
        prep(1)
        soft_scores(0, 0)
        soft_main(0, 0)
        soft_scores(0, 1)
        soft_main(0, 1)
        soft_tr(0, 0)
        soft_tr(0, 1)
        soft_gv(0)
        for i in range(NLOC):
            if i + 1 < NLOC:
                soft_scores(i + 1, 0)
            tail(i)
            if i + 1 < NLOC:
                soft_main(i + 1, 0)
                soft_scores(i + 1, 1)
            if i + 2 < NLOC:
                prep(i + 2)
            if i + 1 < NLOC:
                soft_main(i + 1, 1)
                soft_tr(i + 1, 0)
                soft_tr(i + 1, 1)
                soft_gv(i + 1)

    nc.finalize()
    return nc


_NC_CACHE = {}


def kernel(P, M, mask, Wq, bq, Wk, bk, Wv, bv):
    P = np.ascontiguousarray(P, dtype=np.float32)
    M = np.ascontiguousarray(M, dtype=np.float32)
    maskf = np.where(np.asarray(mask), np.float32(0.0), np.float32(-1e15)).astype(np.float32)
    # permuted on-chip s layout: s = sc*128 + tm*4 + vl  (v = sc*4 + vl)
    maskrow = np.ascontiguousarray(
        np.broadcast_to(
            maskf.reshape(N, SC, 1, 4), (N, SC, mT, 4)
        ).reshape(N, S)
    )
    Wq = np.ascontiguousarray(Wq, dtype=np.float32)
    bq = np.ascontiguousarray(bq, dtype=np.float32)
    Wk = np.ascontiguousarray(Wk, dtype=np.float32)
    Wv = np.ascontiguousarray(Wv, dtype=np.float32)
    bv32 = (32.0 * np.asarray(bv)).astype(np.float32)

    if "nc" not in _NC_CACHE:
        _NC_CACHE["nc"] = _build_graph()
    nc = _NC_CACHE["nc"]

    in_maps = []
    for i in range(NCORES):
        in_maps.append({
            "P": np.ascontiguousarray(P[:, i * NLOC:(i + 1) * NLOC, :]),
            "M": np.ascontiguousarray(M[:, i * NLOC * mV:(i + 1) * NLOC * mV, :]),
            "maskf": np.ascontiguousarray(maskrow[i * NLOC:(i + 1) * NLOC, :]),
            "Wq": Wq, "bq": bq, "Wk": Wk, "Wv": Wv, "bv32": bv32,
        })
    res = run_bass_kernel_spmd(nc, in_maps, core_ids=list(range(NCORES)))
    outs = res.results
    out = np.concatenate([outs[i]["out"] for i in range(NCORES)], axis=1)
    att = np.concatenate([outs[i]["att"] for i in range(NCORES)], axis=0)
    return out.astype(np.float32), att.astype(np.float32)


# revision 32
# speedup vs baseline: 1.7359x; 1.1023x over previous
"""Sparse-attention Trainium2 kernel (8 NeuronCores, data-parallel over N).

Math (per batch element n, derived from the reference):
    Q  = Pn @ Wq.T + bq                      [T, C]
    scores = Q @ K.T  with K = Mn @ Wk.T + bk
           = (Q @ Wk) @ Mn.T + (Q . bk)      -- bk term is constant over the
                                                softmax group (mV axis) -> dropped
    Q' = Q @ Wk                              [T, C]
    sm = scores + additive_mask              (masked lanes == -1e15 exactly in f32)
    attn = softmax over mV groups            (per-t row max as the exp shift --
                                              safe: observed per-row group-max gap
                                              << 87, so no under/overflow)
    out = attn @ V,  V = Mn @ Wv.T + bv
        = (attn @ Mn) @ Wv.T + 32*bv         -- since sum_s attn[t, s] == 32 exactly
    att = mean over (t, tm) of attn          [mV]

Layouts on chip (partition dim first):
    PT   [c', (cc, t)]      via PE transpose of P natural
    QT   [c, (cc, t)]       = WqT.T @ PT    (+bq via ACT bias)
    Q'T  [c', (cc, t)]      = Wk_nat.T @ QT
    Mn   [s, (sc, c)]       natural DMA load (used as lhsT for Z)
    MnT  [c', (cc, s)]      via PE transpose (moving operand for scores)
    scores PSUM [t, s-slice] = Q'T.T @ MnT, + mask via K=1 matmul;
    exp on ACT with bias=-rowmax (fused PSUM->SBUF)
    attn [t, (tc, s)]       normalized on DVE, transposed to attnT [s, (sc, t)]
    ZT   [c', (cc, t)]      = Mn.T @ attnT
    out  [t, (tc, c)]       = ZT.T @ WvT + 32*bv
"""

import sys

for _p in ("/opt/trn_rl_repo",):
    if _p not in sys.path:
        sys.path.insert(0, _p)

from contextlib import ExitStack

import numpy as np

import concourse.bass as bass
from concourse import bacc
import concourse.mybir as mybir
import concourse.tile as tile
from concourse.bass_utils import run_bass_kernel_spmd
from concourse.masks import make_identity

T, N, C = 256, 64, 256
mV, mT = 64, 32
S = mV * mT          # 2048
NCORES = 8
NLOC = N // NCORES   # 8 batch elements per core
TC = T // 128        # 2 t-chunks
CC = C // 128        # 2 c-chunks
SC = S // 128        # 16 s-chunks
SL = S // 512        # 4 s-slices (fp32 moving max)
SG = SC // 4         # 4 transpose groups of 4 chunks

F32 = mybir.dt.float32
F32R = mybir.dt.float32r
BF16 = mybir.dt.bfloat16
AX = mybir.AxisListType
AF = mybir.ActivationFunctionType


def r(ap):
    """Bitcast an AP to float32r for full-rate fp32 matmuls."""
    return ap.bitcast(F32R)


def _build_graph():
    nc = bacc.Bacc()

    P_d = nc.declare_dram_parameter("P", [T, NLOC, C], F32, isOutput=False)
    M_d = nc.declare_dram_parameter("M", [mT, NLOC * mV, C], F32, isOutput=False)
    mk_d = nc.declare_dram_parameter("maskf", [NLOC, S], F32, isOutput=False)
    Wq_d = nc.declare_dram_parameter("Wq", [C, C], F32, isOutput=False)
    bq_d = nc.declare_dram_parameter("bq", [C], F32, isOutput=False)
    Wk_d = nc.declare_dram_parameter("Wk", [C, C], F32, isOutput=False)
    Wv_d = nc.declare_dram_parameter("Wv", [C, C], F32, isOutput=False)
    bv32_d = nc.declare_dram_parameter("bv32", [C], F32, isOutput=False)
    out_d = nc.declare_dram_parameter("out", [T, NLOC, C], F32, isOutput=True)
    att_d = nc.declare_dram_parameter("att", [NLOC, mV], F32, isOutput=True)

    with tile.TileContext(nc) as tc, ExitStack() as ctx:
        const = ctx.enter_context(tc.tile_pool(name="const", bufs=1))
        mn_pool = ctx.enter_context(tc.tile_pool(name="mn", bufs=3))
        mnt_pool = ctx.enter_context(tc.tile_pool(name="mnt", bufs=2))
        attn_pool = ctx.enter_context(tc.tile_pool(name="attn", bufs=2))
        attnt_pool = ctx.enter_context(tc.tile_pool(name="attnt", bufs=2))
        work = ctx.enter_context(tc.tile_pool(name="work", bufs=2))
        outp = ctx.enter_context(tc.tile_pool(name="outp", bufs=3))
        ps_big = ctx.enter_context(tc.tile_pool(name="psbig", bufs=4, space="PSUM"))
        ps_tr = ctx.enter_context(tc.tile_pool(name="pstr", bufs=2, space="PSUM"))
        ps_sm = ctx.enter_context(tc.tile_pool(name="pssm", bufs=2, space="PSUM"))

        # ---- constants ----
        ident = const.tile([128, 128], F32)
        make_identity(nc, ident)
        identb = const.tile([128, 128], BF16)
        make_identity(nc, identb)
        onecol_b = const.tile([128, 1], BF16)
        nc.vector.memset(onecol_b, 1.0)
        ones_raw = const.tile([1, 128], F32)
        nc.vector.memset(ones_raw, 1.0)
        ones1 = const.tile([1, 128], F32)
        nc.scalar.copy(r(ones1), ones_raw)

        # weights: single batched DMA each
        wq_nat = const.tile([128, CC, C], F32)
        wv_nat = const.tile([128, CC, C], F32)
        wk_sb = const.tile([128, CC, C], F32)   # Wk natural [co, (coc, ci)] used as-is
        nc.sync.dma_start(out=wq_nat, in_=Wq_d[:, :].rearrange("(cc p) ci -> p cc ci", p=128))
        nc.sync.dma_start(out=wv_nat, in_=Wv_d[:, :].rearrange("(cc p) ci -> p cc ci", p=128))
        nc.sync.dma_start(out=r(wk_sb), in_=r(Wk_d[:, :].rearrange("(cc p) ci -> p cc ci", p=128)))
        wqt = const.tile([128, CC, C], F32)     # [ci_loc, cic, co]
        wvt = const.tile([128, CC, C], F32)
        for cic in range(CC):
            pst = ps_tr.tile([128, 512], F32, tag="pstr")
            nc.tensor.transpose(pst[:, 0:128], wq_nat[:, 0, cic * 128:(cic + 1) * 128], ident)
            nc.tensor.transpose(pst[:, 128:256], wq_nat[:, 1, cic * 128:(cic + 1) * 128], ident)
            nc.tensor.transpose(pst[:, 256:384], wv_nat[:, 0, cic * 128:(cic + 1) * 128], ident)
            nc.tensor.transpose(pst[:, 384:512], wv_nat[:, 1, cic * 128:(cic + 1) * 128], ident)
            nc.scalar.copy(r(wqt[:, cic, :]), pst[:, 0:256])
            nc.scalar.copy(r(wvt[:, cic, :]), pst[:, 256:512])

        bq_sb = const.tile([128, CC], F32)
        for cc in range(CC):
            nc.sync.dma_start(
                out=bq_sb[:, cc:cc + 1],
                in_=bq_d[cc * 128:(cc + 1) * 128].unsqueeze(1),
            )
        bv32_sb = const.tile([128, C], F32)
        nc.sync.dma_start(out=bv32_sb, in_=bv32_d[:].partition_broadcast(128))

        # ---- software-pipelined per-batch-element stages ----
        # prep(n):  P/M loads, PT/QT/Q'T, MnT transposes  (PE/ACT/DMA)
        # soft(n):  scores, group-max softmax, normalize, attnT  (PE/DVE/ACT)
        # tail(n):  att matmuls, ZT, out  (PE)
        # Emission order interleaves prep(i+2) | soft(i+1)+tail-of-PE work so the
        # PE stream never waits long on the DVE softmax chain.
        state = {}

        def prep(n):
            p_nat = work.tile([128, TC, C], F32, tag="pnat")
            nc.sync.dma_start(
                out=p_nat,
                in_=P_d[:, n, :].rearrange("(tc p) c -> p tc c", p=128),
            )
            pt = work.tile([128, CC, T], F32, tag="pt")
            pstp = ps_tr.tile([128, 512], F32, tag="pstr")
            for tc_i in range(TC):
                for cc in range(CC):
                    nc.tensor.transpose(
                        pstp[:, (cc * TC + tc_i) * 128:(cc * TC + tc_i + 1) * 128],
                        p_nat[:, tc_i, cc * 128:(cc + 1) * 128], ident,
                    )
            for cc in range(CC):
                nc.scalar.copy(r(pt[:, cc, :]), pstp[:, cc * 256:(cc + 1) * 256])

            qt = work.tile([128, CC, T], F32, tag="qt")
            for cc in range(CC):
                psq = ps_sm.tile([128, T], F32, tag="pssm")
                for k in range(CC):
                    nc.tensor.matmul(
                        psq, r(wqt[:, k, cc * 128:(cc + 1) * 128]), r(pt[:, k, :]),
                        start=(k == 0), stop=(k == CC - 1),
                    )
                nc.scalar.add(r(qt[:, cc, :]), psq, add=bq_sb[:, cc:cc + 1])

            qpt = work.tile([128, CC, T], F32, tag="qpt")
            for cc in range(CC):
                psq = ps_sm.tile([128, T], F32, tag="pssm")
                for k in range(CC):
                    nc.tensor.matmul(
                        psq, r(wk_sb[:, k, cc * 128:(cc + 1) * 128]), r(qt[:, k, :]),
                        start=(k == 0), stop=(k == CC - 1),
                    )
                nc.scalar.copy(r(qpt[:, cc, :]), psq)

            # mn partitions are tm-major: p = tm*4 + vl (matches M's DRAM
            # order, giving 32x4KB DMA descriptors instead of 128x1KB)
            mn = mn_pool.tile([128, SC, C], F32, tag="mn")
            for sc in range(SC):
                msrc = M_d[:, n * mV + sc * 4:n * mV + (sc + 1) * 4, :]
                eng = nc.sync if sc % 2 == 0 else nc.scalar
                eng.dma_start(out=r(mn[:, sc, :]), in_=r(msrc))
            mnt = mnt_pool.tile([128, CC, S], F32, tag="mnt")
            for cc in range(CC):
                for g in range(SG):
                    pst = ps_tr.tile([128, 512], F32, tag="pstr")
                    for j in range(4):
                        nc.tensor.transpose(
                            pst[:, j * 128:(j + 1) * 128],
                            mn[:, g * 4 + j, cc * 128:(cc + 1) * 128], ident,
                        )
                    nc.scalar.copy(r(mnt[:, cc, g * 512:(g + 1) * 512]), pst)

            mrow = work.tile([1, S], F32, tag="mrow")
            nc.sync.dma_start(out=r(mrow), in_=r(mk_d[n:n + 1, :]))
            state[n] = dict(qpt=qpt, mn=mn, mnt=mnt, mrow=mrow)

        def soft_scores(n, tc_i):
            st = state[n]
            qpt, mnt, mrow = st["qpt"], st["mnt"], st["mrow"]
            if tc_i == 0:
                attn_sb = attn_pool.tile([128, TC, S], BF16, tag="attn")
                attnt = attnt_pool.tile([128, SC, T], F32, tag="attnt")
                gv = work.tile([128, TC, mV], BF16, tag="gv")
                st["attn_sb"], st["attnt"], st["gv"] = attn_sb, attnt, gv
            slices = []
            for k in range(CC):
                for sl in range(SL):
                    if k == 0:
                        pss = ps_big.tile([128, 512], F32, tag="psbig")
                        slices.append(pss)
                    nc.tensor.matmul(
                        slices[sl],
                        r(qpt[:, k, tc_i * 128:(tc_i + 1) * 128]),
                        r(mnt[:, k, sl * 512:(sl + 1) * 512]),
                        start=(k == 0), stop=False,
                    )
            pm = work.tile([128, SL, mT], F32, tag="pm")
            for sl in range(SL):
                nc.tensor.matmul(
                    slices[sl], r(ones1), r(mrow[:, sl * 512:(sl + 1) * 512]),
                    start=False, stop=True,
                )
                nc.vector.reduce_max(
                    out=pm[:, sl, :],
                    in_=slices[sl][:, :]
                    .rearrange("p (scl tm vl) -> p tm scl vl", tm=mT, vl=4),
                    axis=AX.XY,
                )
            st[("slices", tc_i)] = slices
            st[("pm", tc_i)] = pm

        def soft_main(n, tc_i):
            st = state[n]
            attn_sb = st["attn_sb"]
            slices = st.pop(("slices", tc_i))
            pm = st.pop(("pm", tc_i))
            mx = work.tile([128, mT], F32, tag="mx")
            nc.vector.reduce_max(
                out=mx, in_=pm.rearrange("p sl tm -> p tm sl"), axis=AX.X,
            )
            mx_exp = work.tile([128, 128], F32, tag="mxe")
            nc.vector.tensor_copy(
                mx_exp.rearrange("p (tm vl) -> p tm vl", vl=4),
                mx.unsqueeze(2).broadcast_to([128, mT, 4]),
            )
            sc_f = work.tile([128, S], F32, tag="scf")
            for sl in range(SL):
                nc.vector.tensor_sub(
                    sc_f[:, sl * 512:(sl + 1) * 512]
                    .rearrange("p (scl f) -> p scl f", f=128),
                    slices[sl][:, :].rearrange("p (scl f) -> p scl f", f=128),
                    mx_exp.unsqueeze(1).broadcast_to([128, 4, 128]),
                )
                nc.scalar.activation(
                    attn_sb[:, tc_i, sl * 512:(sl + 1) * 512],
                    sc_f[:, sl * 512:(sl + 1) * 512], AF.Exp,
                    bias=0.0, scale=1.0,
                )
            half = S // 2
            dtree = work.tile([128, half], BF16, tag="dtree")
            nc.vector.tensor_add(
                dtree[:, 0:half],
                attn_sb[:, tc_i, 0:half], attn_sb[:, tc_i, half:S],
            )
            w = half // 2
            while w >= 128:
                nc.vector.tensor_add(
                    dtree[:, 0:w], dtree[:, 0:w], dtree[:, w:2 * w],
                )
                w //= 2
            denom = work.tile([128, mT], F32, tag="denom")
            nc.vector.reduce_sum(
                out=denom,
                in_=dtree[:, 0:128].rearrange("p (tm vl) -> p tm vl", vl=4),
                axis=AX.X,
            )
            recip = work.tile([128, mT], BF16, tag="recip")
            with nc.allow_low_precision(reason="attn weights <= 1; bf16 ok"):
                nc.vector.reciprocal(recip, denom)
            recip_exp = work.tile([128, 128], BF16, tag="recipexp")
            nc.vector.tensor_copy(
                recip_exp.rearrange("p (tm vl) -> p tm vl", vl=4),
                recip.unsqueeze(2).broadcast_to([128, mT, 4]),
            )
            nc.vector.tensor_mul(
                attn_sb[:, tc_i, :].rearrange("p (sc f) -> p sc f", f=128),
                attn_sb[:, tc_i, :].rearrange("p (sc f) -> p sc f", f=128),
                recip_exp.unsqueeze(1).broadcast_to([128, SC, 128]),
            )

        def soft_tr(n, tc_i):
            st = state[n]
            attn_sb, attnt = st["attn_sb"], st["attnt"]
            for g in range(2):
                pstb = ps_big.tile([128, 8, 128], BF16, tag="psbig")
                for j in range(8):
                    nc.tensor.transpose(
                        pstb[:, j, :],
                        attn_sb[:, tc_i, (g * 8 + j) * 128:(g * 8 + j + 1) * 128],
                        identb,
                    )
                nc.scalar.copy(
                    r(attnt[:, g * 8:(g + 1) * 8, tc_i * 128:(tc_i + 1) * 128]),
                    pstb,
                )

        def soft_gv(n):
            st = state[n]
            attn_sb, gv = st["attn_sb"], st["gv"]
            for tc_i in range(TC):
                with nc.allow_low_precision(reason="att partials; bf16 ok"):
                    nc.vector.reduce_sum(
                        out=gv[:, tc_i, :],
                        in_=attn_sb[:, tc_i, :]
                        .rearrange("p (sc tm vl) -> p sc vl tm", tm=mT, vl=4),
                        axis=AX.X,
                    )

        def tail(n):
            st = state.pop(n)
            mn, attnt, gv = st["mn"], st["attnt"], st["gv"]
            # att[v] = (1/(T*mT)) * sum_t gv[t, v]: ones-matmul over partitions
            psa = ps_sm.tile([mV, 1], F32, tag="pssm")
            for tc_i in range(TC):
                nc.tensor.matmul(
                    psa, gv[:, tc_i, :], onecol_b,
                    start=(tc_i == 0), stop=(tc_i == TC - 1),
                )
            att_fin = outp.tile([mV, 1], F32, tag="attfin")
            nc.scalar.mul(att_fin, psa, 1.0 / (T * mT))
            nc.sync.dma_start(out=att_d[n, :].unsqueeze(1), in_=att_fin)

            zt = work.tile([128, CC, T], F32, tag="zt")
            for cc in range(CC):
                psz = ps_sm.tile([128, T], F32, tag="pssm")
                for sc in range(SC):
                    nc.tensor.matmul(
                        psz, r(mn[:, sc, cc * 128:(cc + 1) * 128]), r(attnt[:, sc, :]),
                        start=(sc == 0), stop=(sc == SC - 1),
                    )
                nc.scalar.copy(r(zt[:, cc, :]), psz)

            o_sb = outp.tile([128, TC, C], F32, tag="osb")
            for tc_i in range(TC):
                pso = ps_sm.tile([128, C], F32, tag="pssm")
                for k in range(CC):
                    nc.tensor.matmul(
                        pso, r(zt[:, k, tc_i * 128:(tc_i + 1) * 128]), r(wvt[:, k, :]),
                        start=(k == 0), stop=(k == CC - 1),
                    )
                nc.vector.tensor_add(o_sb[:, tc_i, :], pso, bv32_sb)
            nc.sync.dma_start(
                out=out_d[:, n, :].rearrange("(tc p) c -> p tc c", p=128), in_=o_sb,
            )

        # staggered emission: PE filler work slotted inside each DVE block
        prep(0)
        prep(1)
        soft_scores(0, 0)
        soft_main(0, 0)
        soft_scores(0, 1)
        soft_main(0, 1)
        soft_tr(0, 0)
        soft_tr(0, 1)
        soft_gv(0)
        for i in range(NLOC):
            if i + 1 < NLOC:
                soft_scores(i + 1, 0)
            tail(i)
            if i + 1 < NLOC:
                soft_main(i + 1, 0)
                soft_scores(i + 1, 1)
            if i + 2 < NLOC:
                prep(i + 2)
            if i + 1 < NLOC:
                soft_main(i + 1, 1)
                soft_tr(i + 1, 0)
                soft_tr(i + 1, 1)
                soft_gv(i + 1)

    nc.finalize()
    return nc


_NC_CACHE = {}


def kernel(P, M, mask, Wq, bq, Wk, bk, Wv, bv):
    P = np.ascontiguousarray(P, dtype=np.float32)
    M = np.ascontiguousarray(M, dtype=np.float32)
    maskf = np.where(np.asarray(mask), np.float32(0.0), np.float32(-1e15)).astype(np.float32)
    # permuted on-chip s layout: s = sc*128 + tm*4 + vl  (v = sc*4 + vl)
    maskrow = np.ascontiguousarray(
        np.broadcast_to(
            maskf.reshape(N, SC, 1, 4), (N, SC, mT, 4)
        ).reshape(N, S)
    )
    Wq = np.ascontiguousarray(Wq, dtype=np.float32)
    bq = np.ascontiguousarray(bq, dtype=np.float32)
    Wk = np.ascontiguousarray(Wk, dtype=np.float32)
    Wv = np.ascontiguousarray(Wv, dtype=np.float32)
    bv32 = (32.0 * np.asarray(bv)).astype(np.float32)

    if "nc" not in _NC_CACHE:
        _NC_CACHE["nc"] = _build_graph()
    nc = _NC_CACHE["nc"]

    in_maps = []
    for i in range(NCORES):
        in_maps.append({
            "P": np.ascontiguousarray(P[:, i * NLOC:(i + 1) * NLOC, :]),
            "M": np.ascontiguousarray(M[:, i * NLOC * mV:(i + 1) * NLOC * mV, :]),
            "maskf": np.ascontiguousarray(maskrow[i * NLOC:(i + 1) * NLOC, :]),
            "Wq": Wq, "bq": bq, "Wk": Wk, "Wv": Wv, "bv32": bv32,
        })
    res = run_bass_kernel_spmd(nc, in_maps, core_ids=list(range(NCORES)))
    outs = res.results
    out = np.concatenate([outs[i]["out"] for i in range(NCORES)], axis=1)
    att = np.concatenate([outs[i]["att"] for i in range(NCORES)], axis=0)
    return out.astype(np.float32), att.astype(np.float32)
